# revision 5
# baseline (speedup 1.0000x reference)
"""Trainium2 Bass kernel for a masked transformer block + classifier head.

Sharding: data-parallel over batch across 8 NeuronCores; each core runs the
full block for one batch element (no collectives).

Precision: PE matmuls run in float32r (full PE rate, ~1.5e-4 rel err) on the
q/k/score path; V / attention probabilities / FFN tensors use bf16 — their
rounding is i.i.d. across tokens and washes out in the softmax average and
the final mean-pool over 1024 tokens.

Layout per core (N=1024 tokens, D=512, h=8 heads, dh=64):
  - token-major [128, chunk, D] for residual/LN work,
  - transposed [D-part, N] activations (xnT, QT, KT, attnT, xn2T, g1T) feed
    the PE as lhsT/rhs; LN gamma/beta are applied for free as per-partition
    scalars during the PSUM->SBUF copies that follow the PE transposes,
  - attention: scoresT = k @ q^T per head ([m, n]), exp on ACT (1/8 scale
    fused), then outT = [v | 1]^T @ e^T yields the attention output AND the
    softmax denominator in one PSUM accumulation; denominators are gathered
    via an SBUF->SBUF DMA partition remap and broadcast back across
    partitions with a K=8 indicator matmul.

SBUF pressure is handled with tag-chained tile reuse (xnT->attnT,
QT->xn2T, KT->x2, Vp->x3) plus sequential scoped pools (weight staging ->
qkv weights -> attention e^T buffers -> FFN hidden).
"""

import sys

sys.path.insert(0, '/opt/trn_rl_repo')

from contextlib import ExitStack

import numpy as np

import concourse.bass as bass
import concourse.mybir as mybir
import concourse.tile as tile
from concourse import bacc
from concourse.bass_utils import run_bass_kernel_spmd
from concourse.masks import make_identity

P = 128
N = 1024        # tokens
D = 512         # model dim
F = 2048        # mlp dim
C = 1000        # classes
H = 8           # heads
DH = 64         # head dim
NT = N // P     # 8 token chunks
DC = D // P     # 4 model-dim chunks
FC = F // P     # 16 mlp chunks
SCALE = DH ** -0.5
EPS = 1e-5
N_CORES = 8

F32 = mybir.dt.float32
F32R = mybir.dt.float32r
BF16 = mybir.dt.bfloat16
AF = mybir.ActivationFunctionType
ALU = mybir.AluOpType


def build_bass():
    nc = bacc.Bacc(None, target_bir_lowering=False)

    x_d = nc.dram_tensor('x', [N, D], F32, kind='ExternalInput')
    mask_d = nc.dram_tensor('mask', [N, 1], F32, kind='ExternalInput')
    vec_d = {}
    for nm, sz in [('ln1_g', D), ('ln1_b', D), ('bq', D), ('bk', D), ('bv', D),
                   ('bo', D), ('ln2_g', D), ('ln2_b', D), ('b1', F), ('b2', D),
                   ('lnh_g', D), ('lnh_b', D), ('bh', C)]:
        vec_d[nm] = nc.dram_tensor(nm, [sz], F32, kind='ExternalInput')
    w_d = {nm: nc.dram_tensor(nm, shp, F32, kind='ExternalInput')
           for nm, shp in [('Wq', [D, D]), ('Wk', [D, D]), ('Wv', [D, D]),
                           ('Wo', [D, D]), ('W1', [D, F]), ('W2', [F, D]),
                           ('Wh', [D, C])]}
    out_d = nc.dram_tensor('out', [1, C], F32, kind='ExternalOutput')

    def bcast_ap(handle, n):
        return bass.AP(handle, 0, [[0, P], [1, n]])

    with tile.TileContext(nc) as tc, ExitStack() as top:
        consts = top.enter_context(tc.tile_pool(name='consts', bufs=1))
        wts = top.enter_context(tc.tile_pool(name='wts', bufs=1))
        acts = top.enter_context(tc.tile_pool(name='acts', bufs=1))
        mvp = top.enter_context(tc.tile_pool(name='mv', bufs=4))
        zp = top.enter_context(tc.tile_pool(name='z', bufs=3))

        def ln_chunk(x_ap, eps_ap):
            """LayerNorm stats for a [p, D] chunk -> z = (x-mu)*rstd."""
            pp = x_ap.shape[0]
            mv6 = mvp.tile([P, 6], F32, tag='mv6')
            mv2 = mvp.tile([P, 2], F32, tag='mv2')
            nc.vector.bn_stats(out=mv6[:pp], in_=x_ap)
            nc.vector.bn_aggr(out=mv2[:pp], in_=mv6[:pp])
            nc.scalar.activation(out=mv2[:pp, 1:2], in_=mv2[:pp, 1:2],
                                 func=AF.Sqrt, bias=eps_ap, scale=1.0)
            nc.vector.reciprocal(out=mv2[:pp, 1:2], in_=mv2[:pp, 1:2])
            z = zp.tile([P, D], F32, tag='z')
            nc.vector.tensor_scalar(out=z[:pp], in0=x_ap,
                                    scalar1=mv2[:pp, 0:1], scalar2=mv2[:pp, 1:2],
                                    op0=ALU.subtract, op1=ALU.mult)
            return z

        # ---------------- constants / vectors ----------------
        eps_sb = consts.tile([P, 1], F32)
        nc.vector.memset(eps_sb, EPS)
        ident = consts.tile([P, P], F32)
        make_identity(nc, ident)
        ones_bf = consts.tile([P, 1], BF16)
        nc.vector.memset(ones_bf, 1.0)

        # indicator for softmax-denominator broadcast:
        # ind[h, t*128 + s*64 + i] = (h == 2t + s)
        ind_f = consts.tile([P, P * DC], F32)
        nc.gpsimd.memset(ind_f, 0.0)
        nc.gpsimd.affine_select(
            out=ind_f[:].rearrange('h (t s i) -> h t s i', t=DC, s=2),
            in_=ind_f[:].rearrange('h (t s i) -> h t s i', t=DC, s=2),
            compare_op=ALU.not_equal, fill=1.0, base=0,
            pattern=[[-2, DC], [-1, 2], [0, DH]], channel_multiplier=1)
        ind_r = consts.tile([P, P * DC], F32R)
        nc.vector.tensor_copy(ind_r, ind_f)

        maskT = consts.tile([P, NT], F32)
        nc.sync.dma_start(out=maskT,
                          in_=mask_d[:].rearrange('(c p) o -> p (c o)', p=P))
        vec_pm = {}   # [D]-vectors partition-major: [128, DC]
        for nm in ['ln1_g', 'ln1_b', 'bq', 'bk', 'ln2_g', 'ln2_b',
                   'lnh_g', 'lnh_b']:
            t = consts.tile([P, DC], F32, tag=f'v_{nm}')
            nc.sync.dma_start(out=t,
                              in_=vec_d[nm][:].rearrange('(c p) -> p c', p=P))
            vec_pm[nm] = t
        b1T = consts.tile([P, FC], F32)
        nc.sync.dma_start(out=b1T,
                          in_=vec_d['b1'][:].rearrange('(c p) -> p c', p=P))
        bv_bc = consts.tile([P, D], F32)
        nc.sync.dma_start(out=bv_bc, in_=bcast_ap(vec_d['bv'], D))
        bo_bc = consts.tile([P, D], F32)
        nc.sync.dma_start(out=bo_bc, in_=bcast_ap(vec_d['bo'], D))
        b2_sb = consts.tile([1, D], F32)
        nc.sync.dma_start(out=b2_sb, in_=bass.AP(vec_d['b2'], 0, [[0, 1], [1, D]]))
        bh_sb = consts.tile([1, C], F32)
        nc.sync.dma_start(out=bh_sb, in_=bass.AP(vec_d['bh'], 0, [[0, 1], [1, C]]))

        # long-lived activations; tags chain same-slot reuse across phases
        x_res = acts.tile([P, NT, D], F32, tag='xres')
        rs_sb = acts.tile([8, N], F32, tag='rs')
        recip_r = acts.tile([8, N], F32R, tag='recip')

        with tc.tile_pool(name='wqkv', bufs=1) as wqkv:
            # ------------- weights: DMA + cast to f32r / bf16 -------------
            with tc.tile_pool(name='wst', bufs=2) as wst:
                def load_cast(dst, dram_ap, stage_shape):
                    st = wst.tile(stage_shape, F32, tag='wst')
                    nc.sync.dma_start(out=st, in_=dram_ap)
                    nc.vector.tensor_copy(dst, st)

                wq_r = wqkv.tile([P, DC, D], F32R)
                wk_r = wqkv.tile([P, DC, D], F32R)
                wv_r = wqkv.tile([P, DC, D], F32R)
                wo_r = wts.tile([P, DC, D], F32R)
                for dst, dram in [(wq_r, w_d['Wq']), (wk_r, w_d['Wk']),
                                  (wv_r, w_d['Wv']), (wo_r, w_d['Wo'])]:
                    load_cast(dst, dram[:].rearrange('(c p) m -> p c m', p=P),
                              [P, DC, D])
                w1_bf = wts.tile([P, DC, F], BF16)
                for kc in range(DC):
                    load_cast(w1_bf[:, kc, :], w_d['W1'][kc * P:(kc + 1) * P, :],
                              [P, F])
                w2_bf = wts.tile([P, FC, D], BF16)
                for j in range(4):
                    load_cast(w2_bf[:, j * 4:(j + 1) * 4, :],
                              w_d['W2'][j * 512:(j + 1) * 512, :]
                              .rearrange('(c p) m -> p c m', p=P), [P, 4, D])
                wh_r = wts.tile([P, DC, C], F32R)
                for j in range(2):
                    load_cast(wh_r[:, j * 2:(j + 1) * 2, :],
                              w_d['Wh'][j * 256:(j + 1) * 256, :]
                              .rearrange('(c p) m -> p c m', p=P), [P, 2, C])

            # ------------- phase B: mask + LN1 + transpose -------------
            nc.sync.dma_start(out=x_res,
                              in_=x_d[:].rearrange('(c p) m -> p c m', p=P))
            xnT = acts.tile([P, DC, N], F32R, tag='tA')
            with tc.tile_pool(name='ps_t', bufs=4, space='PSUM') as ps_t:
                for i in range(NT):
                    nc.vector.tensor_scalar_mul(x_res[:, i, :], x_res[:, i, :],
                                                maskT[:, i:i + 1])
                    z = ln_chunk(x_res[:, i, :], eps_sb)
                    # bo pre-add into the residual (ordered after LN reads)
                    nc.gpsimd.tensor_tensor(out=x_res[:, i, :],
                                            in0=x_res[:, i, :], in1=bo_bc,
                                            op=ALU.add)
                    for j in range(DC):
                        pt = ps_t.tile([P, P], F32, tag='pt')
                        nc.tensor.transpose(pt, z[:, j * P:(j + 1) * P], ident)
                        nc.any.tensor_scalar(
                            out=xnT[:, j, i * P:(i + 1) * P], in0=pt,
                            scalar1=vec_pm['ln1_g'][:, j:j + 1],
                            scalar2=vec_pm['ln1_b'][:, j:j + 1],
                            op0=ALU.mult, op1=ALU.add)

            # ------------- phase C: QKV projections -------------
            QT = acts.tile([P, DC, N], F32R, tag='tB')
            KT = acts.tile([P, DC, N], F32R, tag='tC')
            Vp = acts.tile([P, NT, H * (DH + 1)], BF16, tag='tD')
            with tc.tile_pool(name='ps_c', bufs=4, space='PSUM') as ps_c:
                for dst, w_r, bT in [(QT, wq_r, vec_pm['bq']),
                                     (KT, wk_r, vec_pm['bk'])]:
                    for j in range(DC):
                        for nh in range(2):
                            pm = ps_c.tile([P, 512], F32, tag='pc')
                            for kc in range(DC):
                                nc.tensor.matmul(
                                    pm, w_r[:, kc, j * P:(j + 1) * P],
                                    xnT[:, kc, nh * 512:(nh + 1) * 512],
                                    start=(kc == 0), stop=(kc == DC - 1))
                            nc.any.tensor_scalar_add(
                                out=dst[:, j, nh * 512:(nh + 1) * 512],
                                in0=pm, scalar1=bT[:, j:j + 1])
                for i in range(NT):
                    pm = ps_c.tile([P, 512], F32, tag='pc')
                    for kc in range(DC):
                        nc.tensor.matmul(pm, xnT[:, kc, i * P:(i + 1) * P],
                                         wv_r[:, kc, :],
                                         start=(kc == 0), stop=(kc == DC - 1))
                    vrow = Vp[:, i, :].rearrange('p (h c) -> p h c', h=H)
                    nc.vector.tensor_tensor(
                        out=vrow[:, :, 0:DH],
                        in0=pm[:].rearrange('p (h c) -> p h c', h=H),
                        in1=bv_bc[:].rearrange('p (h c) -> p h c', h=H),
                        op=ALU.add)
                    nc.vector.memset(vrow[:, :, DH:DH + 1], 1.0)

        # ------------- phase D: attention -------------
        attnT = acts.tile([P, DC, N], F32R, tag='tA')   # reuses xnT slot
        with tc.tile_pool(name='et', bufs=2) as et_pool, \
             tc.tile_pool(name='rstg', bufs=2) as rstg_pool, \
             tc.tile_pool(name='ps_s', bufs=2, space='PSUM') as ps_s, \
             tc.tile_pool(name='ps_av', bufs=2, space='PSUM') as ps_av:
            for h in range(H):
                p0 = DH * (h % 2)
                hj = h // 2
                eT = et_pool.tile([P, NT, N], BF16, tag='eT')
                for m in range(NT):
                    pss = ps_s.tile([P, N], F32, tag='pss')
                    for nh in range(2):
                        nc.tensor.matmul(
                            pss[:, nh * 512:(nh + 1) * 512],
                            KT[p0:p0 + DH, hj, m * P:(m + 1) * P],
                            QT[p0:p0 + DH, hj, nh * 512:(nh + 1) * 512],
                            start=True, stop=True)
                    nc.scalar.activation(out=eT[:, m, :], in_=pss,
                                         func=AF.Exp, scale=SCALE)
                rstg = rstg_pool.tile([DH + 1, N], F32, tag='rstg')
                for nh in range(2):
                    pav = ps_av.tile([DH + 1, 512], F32, tag='pav')
                    for m in range(NT):
                        nc.tensor.matmul(
                            pav, Vp[:, m, h * (DH + 1):(h + 1) * (DH + 1)],
                            eT[:, m, nh * 512:(nh + 1) * 512],
                            start=(m == 0), stop=(m == NT - 1))
                    nc.any.tensor_copy(
                        attnT[p0:p0 + DH, hj, nh * 512:(nh + 1) * 512],
                        pav[0:DH, :])
                    nc.vector.tensor_copy(
                        rstg[DH:DH + 1, nh * 512:(nh + 1) * 512],
                        pav[DH:DH + 1, :])
                nc.sync.dma_start(out=rs_sb[h:h + 1, :],
                                  in_=rstg[DH:DH + 1, :])

        # ------------- phase E: softmax-norm + Wo + LN2 -------------
        with nc.allow_low_precision(reason='softmax denominator in f32r'):
            nc.vector.reciprocal(out=recip_r, in_=rs_sb)
        x2 = acts.tile([P, NT, D], F32, tag='tC')       # reuses KT slot
        xn2T = acts.tile([P, DC, N], BF16, tag='tB')    # reuses QT slot
        with tc.tile_pool(name='ps_e', bufs=3, space='PSUM') as ps_e, \
             tc.tile_pool(name='ps_eb', bufs=2, space='PSUM') as ps_eb, \
             tc.tile_pool(name='ps_et', bufs=3, space='PSUM') as ps_et:
            for t in range(DC):
                for nh in range(2):
                    pb = ps_eb.tile([P, 512], F32, tag='pb')
                    nc.tensor.matmul(pb, ind_r[0:8, t * P:(t + 1) * P],
                                     recip_r[0:8, nh * 512:(nh + 1) * 512],
                                     start=True, stop=True)
                    sl = attnT[:, t, nh * 512:(nh + 1) * 512]
                    nc.vector.tensor_tensor(out=sl, in0=sl.bitcast(F32),
                                            in1=pb, op=ALU.mult)
            for i in range(NT):
                pm = ps_e.tile([P, 512], F32, tag='pe')
                for kc in range(DC):
                    nc.tensor.matmul(pm, attnT[:, kc, i * P:(i + 1) * P],
                                     wo_r[:, kc, :],
                                     start=(kc == 0), stop=(kc == DC - 1))
                nc.vector.tensor_tensor(out=x2[:, i, :], in0=pm,
                                        in1=x_res[:, i, :], op=ALU.add)
                z = ln_chunk(x2[:, i, :], eps_sb)
                for j in range(DC):
                    pt = ps_et.tile([P, P], F32, tag='pt2')
                    nc.tensor.transpose(pt, z[:, j * P:(j + 1) * P], ident)
                    nc.any.tensor_scalar(
                        out=xn2T[:, j, i * P:(i + 1) * P], in0=pt,
                        scalar1=vec_pm['ln2_g'][:, j:j + 1],
                        scalar2=vec_pm['ln2_b'][:, j:j + 1],
                        op0=ALU.mult, op1=ALU.add)

        # ------------- phase F: FFN + pool + head -------------
        x3_bf = acts.tile([P, NT, D], BF16, tag='tD')   # reuses Vp slot
        with tc.tile_pool(name='p_f', bufs=1) as p_f, \
             tc.tile_pool(name='ps_f', bufs=4, space='PSUM') as ps_f, \
             tc.tile_pool(name='ps_p', bufs=1, space='PSUM') as ps_p:
            g1T = p_f.tile([P, FC, N], BF16)
            for fc in range(FC):
                for nh in range(2):
                    pm = ps_f.tile([P, 512], F32, tag='pf')
                    for kc in range(DC):
                        nc.tensor.matmul(
                            pm, w1_bf[:, kc, fc * P:(fc + 1) * P],
                            xn2T[:, kc, nh * 512:(nh + 1) * 512],
                            start=(kc == 0), stop=(kc == DC - 1))
                    nc.scalar.activation(
                        out=g1T[:, fc, nh * 512:(nh + 1) * 512], in_=pm,
                        func=AF.Gelu_apprx_tanh, bias=b1T[:, fc:fc + 1],
                        scale=1.0)
            for i in range(NT):
                pm = ps_f.tile([P, 512], F32, tag='pf')
                for kc in range(FC):
                    nc.tensor.matmul(pm, g1T[:, kc, i * P:(i + 1) * P],
                                     w2_bf[:, kc, :],
                                     start=(kc == 0), stop=(kc == FC - 1))
                nc.vector.tensor_tensor(out=x3_bf[:, i, :], in0=pm,
                                        in1=x2[:, i, :], op=ALU.add)
            # mean pool over tokens
            pp = ps_p.tile([1, D], F32, tag='pp')
            for i in range(NT):
                nc.tensor.matmul(pp, ones_bf, x3_bf[:, i, :],
                                 start=(i == 0), stop=(i == NT - 1))
            pl = acts.tile([1, D], F32, tag='pl')
            nc.scalar.activation(out=pl, in_=pp, func=AF.Copy, scale=1.0 / N)
            nc.vector.tensor_tensor(out=pl, in0=pl, in1=b2_sb, op=ALU.add)
            # head layernorm on the pooled vector
            zh_full = ln_chunk(pl[0:1, :], eps_sb[0:1, :])
            zT_r = acts.tile([P, DC], F32R, tag='zT')
            for j in range(DC):
                pt = ps_p.tile([P, 1], F32, tag='pth')
                nc.tensor.transpose(pt, zh_full[0:1, j * P:(j + 1) * P],
                                    ident[0:1, 0:1])
                nc.any.tensor_scalar(
                    out=zT_r[:, j:j + 1], in0=pt,
                    scalar1=vec_pm['lnh_g'][:, j:j + 1],
                    scalar2=vec_pm['lnh_b'][:, j:j + 1],
                    op0=ALU.mult, op1=ALU.add)
            out_sb = acts.tile([1, C], F32, tag='osb')
            for half in range(2):
                ph = ps_p.tile([1, 500], F32, tag='ph')
                for j in range(DC):
                    nc.tensor.matmul(
                        ph, zT_r[:, j:j + 1],
                        wh_r[:, j, half * 500:(half + 1) * 500],
                        start=(j == 0), stop=(j == DC - 1))
                nc.vector.tensor_tensor(
                    out=out_sb[:, half * 500:(half + 1) * 500], in0=ph,
                    in1=bh_sb[:, half * 500:(half + 1) * 500], op=ALU.add)
            nc.sync.dma_start(out=out_d[:], in_=out_sb)

    nc.finalize()
    return nc


_NC_CACHE = None


def kernel(**inputs) -> np.ndarray:
    global _NC_CACHE
    if _NC_CACHE is None:
        _NC_CACHE = build_bass()
    nc = _NC_CACHE

    arr = {k: np.ascontiguousarray(np.asarray(v, dtype=np.float32))
           for k, v in inputs.items()}
    x = arr.pop('x')                       # [8, 1024, 512]
    in_maps = [dict(arr, x=np.ascontiguousarray(x[i])) for i in range(N_CORES)]
    res = run_bass_kernel_spmd(nc, in_maps, core_ids=list(range(N_CORES)))
    return np.concatenate([res.results[i]['out'] for i in range(N_CORES)],
                          axis=0)


if __name__ == '__main__':
    rng = np.random.default_rng(0)
    s = lambda d: 1.0 / np.sqrt(d)
    ins = {
        'x': rng.standard_normal((8, N, D), dtype=np.float32),
        'mask': np.ones((N, 1), np.float32),
        'ln1_g': np.ones(D, np.float32), 'ln1_b': np.zeros(D, np.float32),
        'Wq': rng.standard_normal((D, D), dtype=np.float32) * s(D),
        'bq': np.zeros(D, np.float32),
        'Wk': rng.standard_normal((D, D), dtype=np.float32) * s(D),
        'bk': np.zeros(D, np.float32),
        'Wv': rng.standard_normal((D, D), dtype=np.float32) * s(D),
        'bv': np.zeros(D, np.float32),
        'Wo': rng.standard_normal((D, D), dtype=np.float32) * s(D),
        'bo': np.zeros(D, np.float32),
        'ln2_g': np.ones(D, np.float32), 'ln2_b': np.zeros(D, np.float32),
        'W1': rng.standard_normal((D, F), dtype=np.float32) * s(D),
        'b1': np.zeros(F, np.float32),
        'W2': rng.standard_normal((F, D), dtype=np.float32) * s(F),
        'b2': np.zeros(D, np.float32),
        'lnh_g': np.ones(D, np.float32), 'lnh_b': np.zeros(D, np.float32),
        'Wh': rng.standard_normal((D, C), dtype=np.float32) * s(D),
        'bh': np.zeros(C, np.float32),
    }
    out = kernel(**ins)
    print('out', out.shape, out.dtype, float(np.abs(out).max()))


# revision 22
# speedup vs baseline: 18.6971x; 18.6971x over previous
"""Trainium2 Bass kernel for a masked transformer block + classifier head.

Sharding: data-parallel over batch across 8 NeuronCores; each core runs the
full block for one batch element (no collectives).

Precision: PE matmuls run in float32r (full PE rate, ~1.5e-4 rel err) on the
q/k/score path; V / attention probabilities / FFN tensors use bf16 — their
rounding is i.i.d. across tokens and washes out in the softmax average and
the final mean-pool over 1024 tokens.

Layout per core (N=1024 tokens, D=512, h=8 heads, dh=64):
  - token-major [128, D] chunks for residual/LN work,
  - transposed [D-part, N] activations (xnT, QT, KT, attnT, xn2T, g1T) feed
    the PE as lhsT/rhs; LN gamma/beta are applied for free as per-partition
    scalars during the PSUM->SBUF copies that follow the PE transposes,
  - rstd = exp(-0.5*ln(var+eps)) keeps all transcendentals in the single
    natural_log_exp ACT table set (no table thrash with the attention exps),
  - attention: scoresT = k @ q^T per head ([m, n]), exp on ACT (1/8 scale
    fused), then outT = [v | 1]^T @ e^T yields the attention output AND the
    softmax denominator in one PSUM accumulation; denominators are gathered
    via an SBUF->SBUF DMA partition remap and broadcast back across
    partitions with a K=8 indicator matmul.

All big tensors are split into per-chunk tiles so Tile's per-tile dependency
tracking lets downstream matmuls start as soon as their chunk is ready.
SBUF reuse is via tag-chained slots (xnT->attnT, QT->xn2T, KT->x2, Vp->x3)
plus sequential scoped pools (weight staging, e^T buffers, FFN hidden).
"""

import sys

sys.path.insert(0, '/opt/trn_rl_repo')

from contextlib import ExitStack

import numpy as np

import concourse.bass as bass
import concourse.mybir as mybir
import concourse.tile as tile
from concourse import bacc
from concourse.bass_utils import run_bass_kernel_spmd
from concourse.masks import make_identity

P = 128
N = 1024        # tokens
D = 512         # model dim
F = 2048        # mlp dim
C = 1000        # classes
H = 8           # heads
DH = 64         # head dim
NT = N // P     # 8 token chunks
DC = D // P     # 4 model-dim chunks
FC = F // P     # 16 mlp chunks
SCALE = DH ** -0.5
EPS = 1e-5
N_CORES = 8

F32 = mybir.dt.float32
F32R = mybir.dt.float32r
BF16 = mybir.dt.bfloat16
AF = mybir.ActivationFunctionType
ALU = mybir.AluOpType


def _pin_exp_ln_table_set(arch: str):
    """Make Exp and Ln resolve only to the combined natural_log_exp set.

    bacc's table-load inserter greedily picks the first act-table set
    containing each function; Exp alone would pick exp_and_others and Ln
    would pick natural_log, thrashing ~1.3us table loads on every LN<->attn
    alternation. get_activation_tables() is functools.cache'd, so mutating
    the cached dict in place (set IDs = dict order are preserved) pins both
    functions to the one set that holds them together."""
    from concourse.hw_specs import get_activation_tables
    tables = get_activation_tables(arch)
    for name, funcs in tables.items():
        if name == 'natural_log_exp_and_others':
            continue
        funcs.discard(AF.Exp)
        funcs.discard(AF.Ln)


def build_bass():
    nc = bacc.Bacc(None, target_bir_lowering=False)
    _pin_exp_ln_table_set(nc.m.arch)

    x_d = nc.dram_tensor('x', [N, D], F32, kind='ExternalInput')
    mask_d = nc.dram_tensor('mask', [N, 1], F32, kind='ExternalInput')
    vec_d = {}
    for nm, sz in [('ln1_g', D), ('ln1_b', D), ('bq', D), ('bk', D), ('bv', D),
                   ('bo', D), ('ln2_g', D), ('ln2_b', D), ('b1', F), ('b2', D),
                   ('lnh_g', D), ('lnh_b', D), ('bh', C)]:
        vec_d[nm] = nc.dram_tensor(nm, [sz], F32, kind='ExternalInput')
    w_d = {nm: nc.dram_tensor(nm, shp, F32, kind='ExternalInput')
           for nm, shp in [('Wq', [D, D]), ('Wk', [D, D]), ('Wv', [D, D]),
                           ('Wo', [D, D]), ('W1', [D, F]), ('W2', [F, D]),
                           ('Wh', [D, C])]}
    out_d = nc.dram_tensor('out', [1, C], F32, kind='ExternalOutput')

    def bcast_ap(handle, n):
        return bass.AP(handle, 0, [[0, P], [1, n]])

    with tile.TileContext(nc) as tc, ExitStack() as top:
        consts = top.enter_context(tc.tile_pool(name='consts', bufs=1))
        wts = top.enter_context(tc.tile_pool(name='wts', bufs=1))
        acts = top.enter_context(tc.tile_pool(name='acts', bufs=1))
        mvp = top.enter_context(tc.tile_pool(name='mv', bufs=4))
        zp = top.enter_context(tc.tile_pool(name='z', bufs=2))

        def ln_chunk(x_ap, eps_ap):
            """LayerNorm stats for a [p, D] chunk -> z = (x-mu)*rstd.

            rstd = exp(-0.5 * ln(var + eps)) so the only ACT table set used
            anywhere is natural_log_exp (shared with the attention exps)."""
            pp = x_ap.shape[0]
            mv6 = mvp.tile([P, 6], F32, tag='mv6')
            mv2 = mvp.tile([P, 2], F32, tag='mv2')
            nc.vector.bn_stats(out=mv6[:pp], in_=x_ap)
            nc.vector.bn_aggr(out=mv2[:pp], in_=mv6[:pp])
            nc.scalar.activation(out=mv2[:pp, 1:2], in_=mv2[:pp, 1:2],
                                 func=AF.Ln, bias=eps_ap, scale=1.0)
            nc.scalar.activation(out=mv2[:pp, 1:2], in_=mv2[:pp, 1:2],
                                 func=AF.Exp, scale=-0.5)
            z = zp.tile([P, D], F32, tag='z')
            nc.vector.tensor_scalar(out=z[:pp], in0=x_ap,
                                    scalar1=mv2[:pp, 0:1], scalar2=mv2[:pp, 1:2],
                                    op0=ALU.subtract, op1=ALU.mult)
            return z

        # ---------------- constants / vectors ----------------
        eps_sb = consts.tile([P, 1], F32)
        nc.vector.memset(eps_sb, EPS)
        ident = consts.tile([P, P], F32)
        make_identity(nc, ident)
        ones_bf = consts.tile([P, 1], BF16)
        nc.vector.memset(ones_bf, 1.0)

        # indicator for softmax-denominator broadcast:
        # ind[h, t*128 + s*64 + i] = (h == 2t + s)
        ind_f = consts.tile([P, P * DC], F32)
        nc.gpsimd.memset(ind_f, 0.0)
        nc.gpsimd.affine_select(
            out=ind_f[:].rearrange('h (t s i) -> h t s i', t=DC, s=2),
            in_=ind_f[:].rearrange('h (t s i) -> h t s i', t=DC, s=2),
            compare_op=ALU.not_equal, fill=1.0, base=0,
            pattern=[[-2, DC], [-1, 2], [0, DH]], channel_multiplier=1)
        ind_r = consts.tile([P, P * DC], F32R)
        nc.vector.tensor_copy(ind_r, ind_f)

        maskT = consts.tile([P, NT], F32)
        nc.sync.dma_start(out=maskT,
                          in_=mask_d[:].rearrange('(c p) o -> p (c o)', p=P))
        vec_pm = {}   # [D]-vectors partition-major: [128, DC]
        for nm in ['ln1_g', 'ln1_b', 'bq', 'bk', 'ln2_g', 'ln2_b',
                   'lnh_g', 'lnh_b']:
            t = consts.tile([P, DC], F32, tag=f'v_{nm}')
            nc.sync.dma_start(out=t,
                              in_=vec_d[nm][:].rearrange('(c p) -> p c', p=P))
            vec_pm[nm] = t
        b1T = consts.tile([P, FC], F32)
        nc.sync.dma_start(out=b1T,
                          in_=vec_d['b1'][:].rearrange('(c p) -> p c', p=P))
        bv_bc = consts.tile([P, D], F32)
        nc.sync.dma_start(out=bv_bc, in_=bcast_ap(vec_d['bv'], D))
        bo_bc = consts.tile([P, D], F32)
        nc.sync.dma_start(out=bo_bc, in_=bcast_ap(vec_d['bo'], D))

        # long-lived activations (per-chunk tiles; tags chain slot reuse)
        x_res = [acts.tile([P, D], F32, tag=f'xr{i}', name=f'xres{i}') for i in range(NT)]
        rs_sb = acts.tile([8, N], F32, tag='rs')
        recip_r = acts.tile([8, N], F32R, tag='recip')

        # x chunks queued FIRST: the HWDGE queue is FIFO, and LN1 (which
        # gates everything) needs x long before any weight is needed.
        for i in range(NT):
            nc.sync.dma_start(out=x_res[i], in_=x_d[i * P:(i + 1) * P, :])

        with tc.tile_pool(name='wqkv', bufs=1) as wqkv:
            # -- weights: DMA + cast to f32r / bf16, in order of first use --
            with tc.tile_pool(name='wst', bufs=2) as wst:
                def load_cast(dst, dram_ap, stage_shape):
                    st = wst.tile(stage_shape, F32, tag='wst')
                    nc.sync.dma_start(out=st, in_=dram_ap)
                    # DVE, not nc.any: on ACT these 2us casts would queue
                    # ahead of the attention exps (engine queues are FIFO).
                    nc.vector.tensor_copy(dst, st)

                wq_r = wqkv.tile([P, DC, D], F32R)
                wk_r = wqkv.tile([P, DC, D], F32R)
                wv_r = wqkv.tile([P, DC, D], F32R)
                wo_r = wts.tile([P, DC, D], F32R)
                for dst, dram in [(wq_r, w_d['Wq']), (wk_r, w_d['Wk']),
                                  (wv_r, w_d['Wv']), (wo_r, w_d['Wo'])]:
                    load_cast(dst, dram[:].rearrange('(c p) m -> p c m', p=P),
                              [P, DC, D])
                w1_bf = wts.tile([P, DC, F], BF16)
                for kc in range(DC):
                    load_cast(w1_bf[:, kc, :], w_d['W1'][kc * P:(kc + 1) * P, :],
                              [P, F])
                w2_bf = wts.tile([P, FC, D], BF16)
                for j in range(4):
                    load_cast(w2_bf[:, j * 4:(j + 1) * 4, :],
                              w_d['W2'][j * 512:(j + 1) * 512, :]
                              .rearrange('(c p) m -> p c m', p=P), [P, 4, D])
                wh_r = wts.tile([P, DC, C], F32R)
                for j in range(2):
                    load_cast(wh_r[:, j * 2:(j + 1) * 2, :],
                              w_d['Wh'][j * 256:(j + 1) * 256, :]
                              .rearrange('(c p) m -> p c m', p=P), [P, 2, C])

            # ------------- phase B: mask + LN1 + transpose -------------
            # One PSUM pool spans B..D: separate scoped pools would insert
            # address-reuse deps that serialize each phase behind the last
            # PSUM reader of the previous one. Tags: pt x2 + mm x2 + pss 2x2
            # banks = 8 banks exactly.
            es_ps = ExitStack()
            ps_bcd = es_ps.enter_context(tc.tile_pool(name='ps_bcd',
                                                      space='PSUM'))
            xnT = [acts.tile([P, N], F32R, tag=f'tA{j}', name=f'xnT{j}') for j in range(DC)]
            if True:
                for i in range(NT):
                    nc.gpsimd.tensor_scalar_mul(x_res[i], x_res[i],
                                                maskT[:, i:i + 1])
                    z = ln_chunk(x_res[i], eps_sb)
                    # bo pre-add into the residual (ordered after LN reads)
                    nc.gpsimd.tensor_tensor(out=x_res[i], in0=x_res[i],
                                            in1=bo_bc, op=ALU.add)
                    for j in range(DC):
                        pt = ps_bcd.tile([P, P], F32, tag='pt', bufs=2,
                                         name='ptB')
                        nc.tensor.transpose(pt, z[:, j * P:(j + 1) * P], ident)
                        nc.any.tensor_scalar(
                            out=xnT[j][:, i * P:(i + 1) * P], in0=pt,
                            scalar1=vec_pm['ln1_g'][:, j:j + 1],
                            scalar2=vec_pm['ln1_b'][:, j:j + 1],
                            op0=ALU.mult, op1=ALU.add)

            # ------------- phase C: QKV projections -------------
            QT = [acts.tile([P, N], F32R, tag=f'tB{j}', name=f'QT{j}') for j in range(DC)]
            KT = [acts.tile([P, N], F32R, tag=f'tC{j}', name=f'KT{j}') for j in range(DC)]
            Vp = [acts.tile([P, H * (DH + 1)], BF16, tag=f'tD{i}', name=f'Vp{i}')
                  for i in range(NT)]
            if True:
                # V first (head-0 AV needs every Vp chunk), then Q/K
                # interleaved per output chunk so head-0/1 scores (which only
                # need QT[0]/KT[0]) can start early in the QKV phase.
                for i in range(NT):
                    pm = ps_bcd.tile([P, 512], F32, tag='mm', bufs=2,
                                     name='pmV')
                    for kc in range(DC):
                        nc.tensor.matmul(pm, xnT[kc][:, i * P:(i + 1) * P],
                                         wv_r[:, kc, :],
                                         start=(kc == 0), stop=(kc == DC - 1))
                    vrow = Vp[i][:].rearrange('p (h c) -> p h c', h=H)
                    nc.vector.tensor_tensor(
                        out=vrow[:, :, 0:DH],
                        in0=pm[:].rearrange('p (h c) -> p h c', h=H),
                        in1=bv_bc[:].rearrange('p (h c) -> p h c', h=H),
                        op=ALU.add)
                    nc.vector.memset(vrow[:, :, DH:DH + 1], 1.0)
                for j in range(DC):
                    for dst, w_r, bT in [(QT, wq_r, vec_pm['bq']),
                                         (KT, wk_r, vec_pm['bk'])]:
                        for nh in range(2):
                            pm = ps_bcd.tile([P, 512], F32, tag='mm', bufs=2,
                                             name='pmC')
                            for kc in range(DC):
                                nc.tensor.matmul(
                                    pm, w_r[:, kc, j * P:(j + 1) * P],
                                    xnT[kc][:, nh * 512:(nh + 1) * 512],
                                    start=(kc == 0), stop=(kc == DC - 1))
                            nc.any.tensor_scalar_add(
                                out=dst[j][:, nh * 512:(nh + 1) * 512],
                                in0=pm, scalar1=bT[:, j:j + 1])

        # ------------- phase D: attention -------------
        attnT = [acts.tile([P, N], F32R, tag=f'tA{j}', name=f'attnT{j}') for j in range(DC)]
        with tc.tile_pool(name='et', bufs=2) as et_pool, \
             tc.tile_pool(name='rstg', bufs=2) as rstg_pool:
            for h in range(H):
                p0 = DH * (h % 2)
                hj = h // 2
                eT = [et_pool.tile([P, N], BF16, tag=f'e{m}', name=f'eT{m}')
                      for m in range(NT)]
                for m in range(NT):
                    pss = ps_bcd.tile([P, N], F32, tag='pss', bufs=2,
                                      name='pss')
                    for nh in range(2):
                        nc.tensor.matmul(
                            pss[:, nh * 512:(nh + 1) * 512],
                            KT[hj][p0:p0 + DH, m * P:(m + 1) * P],
                            QT[hj][p0:p0 + DH, nh * 512:(nh + 1) * 512],
                            start=True, stop=True)
                    nc.scalar.activation(out=eT[m], in_=pss,
                                         func=AF.Exp, scale=SCALE)
                rstg = rstg_pool.tile([DH + 1, N], F32, tag='rstg')
                for nh in range(2):
                    pav = ps_bcd.tile([DH + 1, 512], F32, tag='mm', bufs=2,
                                      name='pav')
                    for m in range(NT):
                        nc.tensor.matmul(
                            pav, Vp[m][:, h * (DH + 1):(h + 1) * (DH + 1)],
                            eT[m][:, nh * 512:(nh + 1) * 512],
                            start=(m == 0), stop=(m == NT - 1))
                    nc.vector.tensor_copy(
                        attnT[hj][p0:p0 + DH, nh * 512:(nh + 1) * 512],
                        pav[0:DH, :])
                    nc.vector.tensor_copy(
                        rstg[DH:DH + 1, nh * 512:(nh + 1) * 512],
                        pav[DH:DH + 1, :])
                nc.sync.dma_start(out=rs_sb[h:h + 1, :],
                                  in_=rstg[DH:DH + 1, :])

        # ------------- phase E: softmax-norm + Wo + LN2 -------------
        with nc.allow_low_precision(reason='softmax denominator in f32r'):
            nc.vector.reciprocal(out=recip_r, in_=rs_sb)
        # x2 chunks pair into the freed KT slots: x2[i] = x2t[i//2][:, i%2, :]
        x2t = [acts.tile([P, 2, D], F32, tag=f'tC{j}', name=f'x2t{j}') for j in range(DC)]
        x2 = [x2t[i // 2][:, i % 2, :] for i in range(NT)]
        # xn2T split by token half: FFN1's nh=0 matmuls can start once the
        # first four LN2 chunks are done instead of waiting for all eight.
        # nh=0 halves reuse the QT slots, nh=1 halves the (dead) x_res slots.
        xn2T = [[acts.tile([P, 512], BF16, tag=(f'tB{j}' if nh == 0
                                                else f'xr{j}'),
                           name=f'xn2T{j}_{nh}') for nh in range(2)]
                for j in range(DC)]
        for t in range(DC):
            for nh in range(2):
                pb = ps_bcd.tile([P, 512], F32, tag='mm', bufs=2, name='pb')
                nc.tensor.matmul(pb, ind_r[0:8, t * P:(t + 1) * P],
                                 recip_r[0:8, nh * 512:(nh + 1) * 512],
                                 start=True, stop=True)
                sl = attnT[t][:, nh * 512:(nh + 1) * 512]
                nc.vector.tensor_tensor(out=sl, in0=sl.bitcast(F32),
                                        in1=pb, op=ALU.mult)
        es_ps.close()
        es_ps2 = ExitStack()
        ps_ef = es_ps2.enter_context(tc.tile_pool(name='ps_ef', space='PSUM'))
        if True:
            for i in range(NT):
                pm = ps_ef.tile([P, 512], F32, tag='mm', bufs=3, name='pmWo')
                for kc in range(DC):
                    nc.tensor.matmul(pm, attnT[kc][:, i * P:(i + 1) * P],
                                     wo_r[:, kc, :],
                                     start=(kc == 0), stop=(kc == DC - 1))
                nc.vector.tensor_tensor(out=x2[i], in0=pm,
                                        in1=x_res[i], op=ALU.add)
                z = ln_chunk(x2[i], eps_sb, z_engine='act')
                for j in range(DC):
                    pt = ps_ef.tile([P, P], F32, tag='pt2', bufs=2, name='ptE')
                    nc.tensor.transpose(pt, z[:, j * P:(j + 1) * P], ident)
                    nc.any.tensor_scalar(
                        out=xn2T[j][i // 4][:, (i % 4) * P:(i % 4 + 1) * P],
                        in0=pt,
                        scalar1=vec_pm['ln2_g'][:, j:j + 1],
                        scalar2=vec_pm['ln2_b'][:, j:j + 1],
                        op0=ALU.mult, op1=ALU.add)

        # ------------- phase F: FFN + pool + head -------------
        x3_bf = [acts.tile([P, D], BF16, tag=f'tD{i}', name=f'x3bf{i}') for i in range(NT)]
        with tc.tile_pool(name='p_f', bufs=1) as p_f:
            b2_sb = p_f.tile([1, D], F32)
            nc.sync.dma_start(out=b2_sb,
                              in_=bass.AP(vec_d['b2'], 0, [[0, 1], [1, D]]))
            bh_sb = p_f.tile([1, C], F32)
            nc.sync.dma_start(out=bh_sb,
                              in_=bass.AP(vec_d['bh'], 0, [[0, 1], [1, C]]))
            g1T = [p_f.tile([P, N], BF16, tag=f'g{fc}', name=f'g1T{fc}') for fc in range(FC)]
            for fc in range(FC):
                for nh in range(2):
                    pm = ps_ef.tile([P, 512], F32, tag='mm', bufs=3,
                                    name='pmF1')
                    for kc in range(DC):
                        nc.tensor.matmul(
                            pm, w1_bf[:, kc, fc * P:(fc + 1) * P],
                            xn2T[kc][nh],
                            start=(kc == 0), stop=(kc == DC - 1))
                    nc.scalar.activation(
                        out=g1T[fc][:, nh * 512:(nh + 1) * 512], in_=pm,
                        func=AF.Gelu_apprx_tanh, bias=b1T[:, fc:fc + 1],
                        scale=1.0)
            for i in range(NT):
                pm = ps_ef.tile([P, 512], F32, tag='mm', bufs=3, name='pmF2')
                for kc in range(FC):
                    nc.tensor.matmul(pm, g1T[kc][:, i * P:(i + 1) * P],
                                     w2_bf[:, kc, :],
                                     start=(kc == 0), stop=(kc == FC - 1))
                nc.vector.tensor_tensor(out=x3_bf[i], in0=pm,
                                        in1=x2[i], op=ALU.add)
            # mean pool over tokens
            pp = ps_ef.tile([1, D], F32, tag='sm', bufs=2, name='pp')
            for i in range(NT):
                nc.tensor.matmul(pp, ones_bf, x3_bf[i],
                                 start=(i == 0), stop=(i == NT - 1))
            pl = zp.tile([P, D], F32, tag='z', name='pl')
            pl = pl[0:1, :]
            nc.scalar.activation(out=pl, in_=pp, func=AF.Copy, scale=1.0 / N)
            nc.vector.tensor_tensor(out=pl, in0=pl, in1=b2_sb, op=ALU.add)
            # head layernorm on the pooled vector
            zh_full = ln_chunk(pl[0:1, :], eps_sb[0:1, :])
            zT_r = acts.tile([P, DC], F32R, tag='zT')
            for j in range(DC):
                pt = ps_ef.tile([P, 1], F32, tag='sm', bufs=2, name='pth')
                nc.tensor.transpose(pt, zh_full[0:1, j * P:(j + 1) * P],
                                    ident[0:1, 0:1])
                nc.any.tensor_scalar(
                    out=zT_r[:, j:j + 1], in0=pt,
                    scalar1=vec_pm['lnh_g'][:, j:j + 1],
                    scalar2=vec_pm['lnh_b'][:, j:j + 1],
                    op0=ALU.mult, op1=ALU.add)
            out_sb = p_f.tile([1, C], F32, tag='osb')
            for half in range(2):
                ph = ps_ef.tile([1, 500], F32, tag='sm', bufs=2, name='ph')
                for j in range(DC):
                    nc.tensor.matmul(
                        ph, zT_r[:, j:j + 1],
                        wh_r[:, j, half * 500:(half + 1) * 500],
                        start=(j == 0), stop=(j == DC - 1))
                nc.vector.tensor_tensor(
                    out=out_sb[:, half * 500:(half + 1) * 500], in0=ph,
                    in1=bh_sb[:, half * 500:(half + 1) * 500], op=ALU.add)
            nc.sync.dma_start(out=out_d[:], in_=out_sb)
        es_ps2.close()

    nc.finalize()
    return nc


_NC_CACHE = None


def kernel(**inputs) -> np.ndarray:
    global _NC_CACHE
    if _NC_CACHE is None:
        _NC_CACHE = build_bass()
    nc = _NC_CACHE

    arr = {k: np.ascontiguousarray(np.asarray(v, dtype=np.float32))
           for k, v in inputs.items()}
    x = arr.pop('x')                       # [8, 1024, 512]
    in_maps = [dict(arr, x=np.ascontiguousarray(x[i])) for i in range(N_CORES)]
    res = run_bass_kernel_spmd(nc, in_maps, core_ids=list(range(N_CORES)))
    return np.concatenate([res.results[i]['out'] for i in range(N_CORES)],
                          axis=0)


if __name__ == '__main__':
    rng = np.random.default_rng(0)
    s = lambda d: 1.0 / np.sqrt(d)
    ins = {
        'x': rng.standard_normal((8, N, D), dtype=np.float32),
        'mask': np.ones((N, 1), np.float32),
        'ln1_g': np.ones(D, np.float32), 'ln1_b': np.zeros(D, np.float32),
        'Wq': rng.standard_normal((D, D), dtype=np.float32) * s(D),
        'bq': np.zeros(D, np.float32),
        'Wk': rng.standard_normal((D, D), dtype=np.float32) * s(D),
        'bk': np.zeros(D, np.float32),
        'Wv': rng.standard_normal((D, D), dtype=np.float32) * s(D),
        'bv': np.zeros(D, np.float32),
        'Wo': rng.standard_normal((D, D), dtype=np.float32) * s(D),
        'bo': np.zeros(D, np.float32),
        'ln2_g': np.ones(D, np.float32), 'ln2_b': np.zeros(D, np.float32),
        'W1': rng.standard_normal((D, F), dtype=np.float32) * s(D),
        'b1': np.zeros(F, np.float32),
        'W2': rng.standard_normal((F, D), dtype=np.float32) * s(F),
        'b2': np.zeros(D, np.float32),
        'lnh_g': np.ones(D, np.float32), 'lnh_b': np.zeros(D, np.float32),
        'Wh': rng.standard_normal((D, C), dtype=np.float32) * s(D),
        'bh': np.zeros(C, np.float32),
    }
    out = kernel(**ins)
    print('out', out.shape, out.dtype, float(np.abs(out).max()))


# revision 23
# speedup vs baseline: 55.3358x; 2.9596x over previous
"""Trainium2 Bass kernel for a masked transformer block + classifier head.

Sharding: data-parallel over batch across 8 NeuronCores; each core runs the
full block for one batch element (no collectives).

Precision: PE matmuls run in float32r (full PE rate, ~1.5e-4 rel err) on the
q/k/score path; V / attention probabilities / FFN tensors use bf16 — their
rounding is i.i.d. across tokens and washes out in the softmax average and
the final mean-pool over 1024 tokens.

Layout per core (N=1024 tokens, D=512, h=8 heads, dh=64):
  - token-major [128, D] chunks for residual/LN work,
  - transposed [D-part, N] activations (xnT, QT, KT, attnT, xn2T, g1T) feed
    the PE as lhsT/rhs; LN gamma/beta are applied for free as per-partition
    scalars during the PSUM->SBUF copies that follow the PE transposes,
  - rstd = exp(-0.5*ln(var+eps)) keeps all transcendentals in the single
    natural_log_exp ACT table set (no table thrash with the attention exps),
  - attention: scoresT = k @ q^T per head ([m, n]), exp on ACT (1/8 scale
    fused), then outT = [v | 1]^T @ e^T yields the attention output AND the
    softmax denominator in one PSUM accumulation; denominators are gathered
    via an SBUF->SBUF DMA partition remap and broadcast back across
    partitions with a K=8 indicator matmul.

All big tensors are split into per-chunk tiles so Tile's per-tile dependency
tracking lets downstream matmuls start as soon as their chunk is ready.
SBUF reuse is via tag-chained slots (xnT->attnT, QT->xn2T, KT->x2, Vp->x3)
plus sequential scoped pools (weight staging, e^T buffers, FFN hidden).
"""

import sys

sys.path.insert(0, '/opt/trn_rl_repo')

from contextlib import ExitStack

import numpy as np

import concourse.bass as bass
import concourse.mybir as mybir
import concourse.tile as tile
from concourse import bacc
from concourse.bass_utils import run_bass_kernel_spmd
from concourse.masks import make_identity

P = 128
N = 1024        # tokens
D = 512         # model dim
F = 2048        # mlp dim
C = 1000        # classes
H = 8           # heads
DH = 64         # head dim
NT = N // P     # 8 token chunks
DC = D // P     # 4 model-dim chunks
FC = F // P     # 16 mlp chunks
SCALE = DH ** -0.5
EPS = 1e-5
N_CORES = 8

F32 = mybir.dt.float32
F32R = mybir.dt.float32r
BF16 = mybir.dt.bfloat16
AF = mybir.ActivationFunctionType
ALU = mybir.AluOpType


def _pin_exp_ln_table_set(arch: str):
    """Make Exp and Ln resolve only to the combined natural_log_exp set.

    bacc's table-load inserter greedily picks the first act-table set
    containing each function; Exp alone would pick exp_and_others and Ln
    would pick natural_log, thrashing ~1.3us table loads on every LN<->attn
    alternation. get_activation_tables() is functools.cache'd, so mutating
    the cached dict in place (set IDs = dict order are preserved) pins both
    functions to the one set that holds them together."""
    from concourse.hw_specs import get_activation_tables
    tables = get_activation_tables(arch)
    for name, funcs in tables.items():
        if name == 'natural_log_exp_and_others':
            continue
        funcs.discard(AF.Exp)
        funcs.discard(AF.Ln)


def build_bass():
    nc = bacc.Bacc(None, target_bir_lowering=False)
    _pin_exp_ln_table_set(nc.m.arch)

    x_d = nc.dram_tensor('x', [N, D], F32, kind='ExternalInput')
    mask_d = nc.dram_tensor('mask', [N, 1], F32, kind='ExternalInput')
    vec_d = {}
    for nm, sz in [('ln1_g', D), ('ln1_b', D), ('bq', D), ('bk', D), ('bv', D),
                   ('bo', D), ('ln2_g', D), ('ln2_b', D), ('b1', F), ('b2', D),
                   ('lnh_g', D), ('lnh_b', D), ('bh', C)]:
        vec_d[nm] = nc.dram_tensor(nm, [sz], F32, kind='ExternalInput')
    w_d = {nm: nc.dram_tensor(nm, shp, F32, kind='ExternalInput')
           for nm, shp in [('Wq', [D, D]), ('Wk', [D, D]), ('Wv', [D, D]),
                           ('Wo', [D, D]), ('W1', [D, F]), ('W2', [F, D]),
                           ('Wh', [D, C])]}
    out_d = nc.dram_tensor('out', [1, C], F32, kind='ExternalOutput')

    def bcast_ap(handle, n):
        return bass.AP(handle, 0, [[0, P], [1, n]])

    with tile.TileContext(nc) as tc, ExitStack() as top:
        consts = top.enter_context(tc.tile_pool(name='consts', bufs=1))
        wts = top.enter_context(tc.tile_pool(name='wts', bufs=1))
        acts = top.enter_context(tc.tile_pool(name='acts', bufs=1))
        mvp = top.enter_context(tc.tile_pool(name='mv', bufs=4))
        zp = top.enter_context(tc.tile_pool(name='z', bufs=2))

        def ln_chunk(x_ap, eps_ap):
            """LayerNorm stats for a [p, D] chunk -> z = (x-mu)*rstd.

            rstd = exp(-0.5 * ln(var + eps)) so the only ACT table set used
            anywhere is natural_log_exp (shared with the attention exps)."""
            pp = x_ap.shape[0]
            mv6 = mvp.tile([P, 6], F32, tag='mv6')
            mv2 = mvp.tile([P, 2], F32, tag='mv2')
            nc.vector.bn_stats(out=mv6[:pp], in_=x_ap)
            nc.vector.bn_aggr(out=mv2[:pp], in_=mv6[:pp])
            nc.scalar.activation(out=mv2[:pp, 1:2], in_=mv2[:pp, 1:2],
                                 func=AF.Ln, bias=eps_ap, scale=1.0)
            nc.scalar.activation(out=mv2[:pp, 1:2], in_=mv2[:pp, 1:2],
                                 func=AF.Exp, scale=-0.5)
            z = zp.tile([P, D], F32, tag='z')
            nc.vector.tensor_scalar(out=z[:pp], in0=x_ap,
                                    scalar1=mv2[:pp, 0:1], scalar2=mv2[:pp, 1:2],
                                    op0=ALU.subtract, op1=ALU.mult)
            return z

        # ---------------- constants / vectors ----------------
        eps_sb = consts.tile([P, 1], F32)
        nc.vector.memset(eps_sb, EPS)
        ident = consts.tile([P, P], F32)
        make_identity(nc, ident)
        ones_bf = consts.tile([P, 1], BF16)
        nc.vector.memset(ones_bf, 1.0)

        # indicator for softmax-denominator broadcast:
        # ind[h, t*128 + s*64 + i] = (h == 2t + s)
        ind_f = consts.tile([P, P * DC], F32)
        nc.gpsimd.memset(ind_f, 0.0)
        nc.gpsimd.affine_select(
            out=ind_f[:].rearrange('h (t s i) -> h t s i', t=DC, s=2),
            in_=ind_f[:].rearrange('h (t s i) -> h t s i', t=DC, s=2),
            compare_op=ALU.not_equal, fill=1.0, base=0,
            pattern=[[-2, DC], [-1, 2], [0, DH]], channel_multiplier=1)
        ind_r = consts.tile([P, P * DC], F32R)
        nc.vector.tensor_copy(ind_r, ind_f)

        maskT = consts.tile([P, NT], F32)
        nc.sync.dma_start(out=maskT,
                          in_=mask_d[:].rearrange('(c p) o -> p (c o)', p=P))
        msqT = consts.tile([P, NT], F32)
        nc.vector.tensor_tensor(out=msqT, in0=maskT, in1=maskT, op=ALU.mult)
        vec_pm = {}   # [D]-vectors partition-major: [128, DC]
        for nm in ['ln1_g', 'ln1_b', 'bq', 'bk', 'ln2_g', 'ln2_b',
                   'lnh_g', 'lnh_b']:
            t = consts.tile([P, DC], F32, tag=f'v_{nm}')
            nc.sync.dma_start(out=t,
                              in_=vec_d[nm][:].rearrange('(c p) -> p c', p=P))
            vec_pm[nm] = t
        b1T = consts.tile([P, FC], F32)
        nc.sync.dma_start(out=b1T,
                          in_=vec_d['b1'][:].rearrange('(c p) -> p c', p=P))
        bv_bc = consts.tile([P, D], F32)
        nc.sync.dma_start(out=bv_bc, in_=bcast_ap(vec_d['bv'], D))
        bo_bc = consts.tile([P, D], F32)
        nc.sync.dma_start(out=bo_bc, in_=bcast_ap(vec_d['bo'], D))

        # long-lived activations (per-chunk tiles; tags chain slot reuse)
        x_res = [acts.tile([P, D], F32, tag=f'xr{i}', name=f'xres{i}') for i in range(NT)]
        rs_sb = acts.tile([8, N], F32, tag='rs')
        recip_r = acts.tile([8, N], F32R, tag='recip')

        # x chunks queued FIRST: the HWDGE queue is FIFO, and LN1 (which
        # gates everything) needs x long before any weight is needed.
        for i in range(NT):
            nc.sync.dma_start(out=x_res[i], in_=x_d[i * P:(i + 1) * P, :])

        with tc.tile_pool(name='wqkv', bufs=1) as wqkv:
            # -- weights: DMA + cast to f32r / bf16, in order of first use --
            with tc.tile_pool(name='wst', bufs=2) as wst:
                def load_cast(dst, dram_ap, stage_shape):
                    st = wst.tile(stage_shape, F32, tag='wst')
                    nc.sync.dma_start(out=st, in_=dram_ap)
                    # DVE, not nc.any: on ACT these 2us casts would queue
                    # ahead of the attention exps (engine queues are FIFO).
                    nc.vector.tensor_copy(dst, st)

                wq_r = wqkv.tile([P, DC, D], F32R)
                wk_r = wqkv.tile([P, DC, D], F32R)
                wv_r = wqkv.tile([P, DC, D], F32R)
                wo_r = wts.tile([P, DC, D], F32R)
                for dst, dram in [(wq_r, w_d['Wq']), (wk_r, w_d['Wk']),
                                  (wv_r, w_d['Wv']), (wo_r, w_d['Wo'])]:
                    load_cast(dst, dram[:].rearrange('(c p) m -> p c m', p=P),
                              [P, DC, D])
                w1_bf = wts.tile([P, DC, F], BF16)
                for kc in range(DC):
                    load_cast(w1_bf[:, kc, :], w_d['W1'][kc * P:(kc + 1) * P, :],
                              [P, F])
                w2_bf = wts.tile([P, FC, D], BF16)
                for j in range(4):
                    load_cast(w2_bf[:, j * 4:(j + 1) * 4, :],
                              w_d['W2'][j * 512:(j + 1) * 512, :]
                              .rearrange('(c p) m -> p c m', p=P), [P, 4, D])
                wh_r = wts.tile([P, DC, C], F32R)
                for j in range(2):
                    load_cast(wh_r[:, j * 2:(j + 1) * 2, :],
                              w_d['Wh'][j * 256:(j + 1) * 256, :]
                              .rearrange('(c p) m -> p c m', p=P), [P, 2, C])

            # ------------- phase B: mask + LN1 + transpose -------------
            # One PSUM pool spans B..D: separate scoped pools would insert
            # address-reuse deps that serialize each phase behind the last
            # PSUM reader of the previous one. Tags: pt x2 + mm x2 + pss 2x2
            # banks = 8 banks exactly.
            es_ps = ExitStack()
            ps_bcd = es_ps.enter_context(tc.tile_pool(name='ps_bcd',
                                                      space='PSUM'))
            xnT = [acts.tile([P, N], F32R, tag=f'tA{j}', name=f'xnT{j}') for j in range(DC)]
            if True:
                for i in range(NT):
                    # LN1 stats on the RAW x chunk; the mask folds in as
                    # var' = m^2 var (Ln scale AP) and z = (x-mu)*(m*rstd'),
                    # keeping the mask multiply off the critical chain.
                    mv6 = mvp.tile([P, 6], F32, tag='mv6', name='mv6b')
                    mv2 = mvp.tile([P, 2], F32, tag='mv2', name='mv2b')
                    nc.vector.bn_stats(out=mv6, in_=x_res[i])
                    nc.vector.bn_aggr(out=mv2, in_=mv6)
                    nc.scalar.activation(out=mv2[:, 1:2], in_=mv2[:, 1:2],
                                         func=AF.Ln, bias=eps_sb,
                                         scale=msqT[:, i:i + 1])
                    nc.scalar.activation(out=mv2[:, 1:2], in_=mv2[:, 1:2],
                                         func=AF.Exp, scale=-0.5)
                    nc.vector.tensor_scalar_mul(mv2[:, 1:2], mv2[:, 1:2],
                                                maskT[:, i:i + 1])
                    z = zp.tile([P, D], F32, tag='z', name='zb')
                    nc.vector.tensor_scalar(out=z, in0=x_res[i],
                                            scalar1=mv2[:, 0:1],
                                            scalar2=mv2[:, 1:2],
                                            op0=ALU.subtract, op1=ALU.mult)
                    # residual = m*x + bo, off the critical path on gpsimd
                    # (WAR-ordered after the raw-x reads above)
                    nc.gpsimd.tensor_scalar_mul(x_res[i], x_res[i],
                                                maskT[:, i:i + 1])
                    nc.gpsimd.tensor_tensor(out=x_res[i], in0=x_res[i],
                                            in1=bo_bc, op=ALU.add)
                    for j in range(DC):
                        pt = ps_bcd.tile([P, P], F32, tag='pt', bufs=2,
                                         name='ptB')
                        nc.tensor.transpose(pt, z[:, j * P:(j + 1) * P], ident)
                        nc.any.tensor_scalar(
                            out=xnT[j][:, i * P:(i + 1) * P], in0=pt,
                            scalar1=vec_pm['ln1_g'][:, j:j + 1],
                            scalar2=vec_pm['ln1_b'][:, j:j + 1],
                            op0=ALU.mult, op1=ALU.add)

            # ------------- phase C: QKV projections -------------
            QT = [acts.tile([P, N], F32R, tag=f'tB{j}', name=f'QT{j}') for j in range(DC)]
            KT = [acts.tile([P, N], F32R, tag=f'tC{j}', name=f'KT{j}') for j in range(DC)]
            Vp = [acts.tile([P, H * (DH + 1)], BF16, tag=f'tD{i}', name=f'Vp{i}')
                  for i in range(NT)]
            if True:
                # V first (head-0 AV needs every Vp chunk), then Q/K
                # interleaved per output chunk so head-0/1 scores (which only
                # need QT[0]/KT[0]) can start early in the QKV phase.
                for i in range(NT):
                    pm = ps_bcd.tile([P, 512], F32, tag='mm', bufs=2,
                                     name='pmV')
                    for kc in range(DC):
                        nc.tensor.matmul(pm, xnT[kc][:, i * P:(i + 1) * P],
                                         wv_r[:, kc, :],
                                         start=(kc == 0), stop=(kc == DC - 1))
                    vrow = Vp[i][:].rearrange('p (h c) -> p h c', h=H)
                    nc.vector.tensor_tensor(
                        out=vrow[:, :, 0:DH],
                        in0=pm[:].rearrange('p (h c) -> p h c', h=H),
                        in1=bv_bc[:].rearrange('p (h c) -> p h c', h=H),
                        op=ALU.add)
                    nc.vector.memset(vrow[:, :, DH:DH + 1], 1.0)
                for j in range(DC):
                    for dst, w_r, bT in [(QT, wq_r, vec_pm['bq']),
                                         (KT, wk_r, vec_pm['bk'])]:
                        for nh in range(2):
                            pm = ps_bcd.tile([P, 512], F32, tag='mm', bufs=2,
                                             name='pmC')
                            for kc in range(DC):
                                nc.tensor.matmul(
                                    pm, w_r[:, kc, j * P:(j + 1) * P],
                                    xnT[kc][:, nh * 512:(nh + 1) * 512],
                                    start=(kc == 0), stop=(kc == DC - 1))
                            nc.any.tensor_scalar_add(
                                out=dst[j][:, nh * 512:(nh + 1) * 512],
                                in0=pm, scalar1=bT[:, j:j + 1])

        # ------------- phase D: attention -------------
        attnT = [acts.tile([P, N], F32R, tag=f'tA{j}', name=f'attnT{j}') for j in range(DC)]
        with tc.tile_pool(name='et', bufs=2) as et_pool, \
             tc.tile_pool(name='rstg', bufs=2) as rstg_pool:
            for h in range(H):
                p0 = DH * (h % 2)
                hj = h // 2
                eT = [et_pool.tile([P, N], BF16, tag=f'e{m}', name=f'eT{m}')
                      for m in range(NT)]
                for m in range(NT):
                    pss = ps_bcd.tile([P, N], F32, tag='pss', bufs=2,
                                      name='pss')
                    for nh in range(2):
                        nc.tensor.matmul(
                            pss[:, nh * 512:(nh + 1) * 512],
                            KT[hj][p0:p0 + DH, m * P:(m + 1) * P],
                            QT[hj][p0:p0 + DH, nh * 512:(nh + 1) * 512],
                            start=True, stop=True)
                    nc.scalar.activation(out=eT[m], in_=pss,
                                         func=AF.Exp, scale=SCALE)
                rstg = rstg_pool.tile([DH + 1, N], F32, tag='rstg')
                for nh in range(2):
                    pav = ps_bcd.tile([DH + 1, 512], F32, tag='mm', bufs=2,
                                      name='pav')
                    for m in range(NT):
                        nc.tensor.matmul(
                            pav, Vp[m][:, h * (DH + 1):(h + 1) * (DH + 1)],
                            eT[m][:, nh * 512:(nh + 1) * 512],
                            start=(m == 0), stop=(m == NT - 1))
                    nc.vector.tensor_copy(
                        attnT[hj][p0:p0 + DH, nh * 512:(nh + 1) * 512],
                        pav[0:DH, :])
                    nc.vector.tensor_copy(
                        rstg[DH:DH + 1, nh * 512:(nh + 1) * 512],
                        pav[DH:DH + 1, :])
                nc.sync.dma_start(out=rs_sb[h:h + 1, :],
                                  in_=rstg[DH:DH + 1, :])

        # ------------- phase E: softmax-norm + Wo + LN2 -------------
        with nc.allow_low_precision(reason='softmax denominator in f32r'):
            nc.vector.reciprocal(out=recip_r, in_=rs_sb)
        # x2 chunks pair into the freed KT slots: x2[i] = x2t[i//2][:, i%2, :]
        x2t = [acts.tile([P, 2, D], F32, tag=f'tC{j}', name=f'x2t{j}') for j in range(DC)]
        x2 = [x2t[i // 2][:, i % 2, :] for i in range(NT)]
        # xn2T split by token half: FFN1's nh=0 matmuls can start once the
        # first four LN2 chunks are done instead of waiting for all eight.
        # nh=0 halves reuse the QT slots, nh=1 halves the (dead) x_res slots.
        xn2T = [[acts.tile([P, 512], BF16, tag=(f'tB{j}' if nh == 0
                                                else f'xr{j}'),
                           name=f'xn2T{j}_{nh}') for nh in range(2)]
                for j in range(DC)]
        for t in range(DC):
            for nh in range(2):
                pb = ps_bcd.tile([P, 512], F32, tag='mm', bufs=2, name='pb')
                nc.tensor.matmul(pb, ind_r[0:8, t * P:(t + 1) * P],
                                 recip_r[0:8, nh * 512:(nh + 1) * 512],
                                 start=True, stop=True)
                sl = attnT[t][:, nh * 512:(nh + 1) * 512]
                nc.vector.tensor_tensor(out=sl, in0=sl.bitcast(F32),
                                        in1=pb, op=ALU.mult)
        es_ps.close()
        es_ps2 = ExitStack()
        ps_ef = es_ps2.enter_context(tc.tile_pool(name='ps_ef', space='PSUM'))
        if True:
            for i in range(NT):
                pm = ps_ef.tile([P, 512], F32, tag='mm', bufs=3, name='pmWo')
                for kc in range(DC):
                    nc.tensor.matmul(pm, attnT[kc][:, i * P:(i + 1) * P],
                                     wo_r[:, kc, :],
                                     start=(kc == 0), stop=(kc == DC - 1))
                nc.vector.tensor_tensor(out=x2[i], in0=pm,
                                        in1=x_res[i], op=ALU.add)
                z = ln_chunk(x2[i], eps_sb, z_engine='act')
                for j in range(DC):
                    pt = ps_ef.tile([P, P], F32, tag='pt2', bufs=2, name='ptE')
                    nc.tensor.transpose(pt, z[:, j * P:(j + 1) * P], ident)
                    nc.any.tensor_scalar(
                        out=xn2T[j][i // 4][:, (i % 4) * P:(i % 4 + 1) * P],
                        in0=pt,
                        scalar1=vec_pm['ln2_g'][:, j:j + 1],
                        scalar2=vec_pm['ln2_b'][:, j:j + 1],
                        op0=ALU.mult, op1=ALU.add)

        # ------------- phase F: FFN + pool + head -------------
        x3_bf = [acts.tile([P, D], BF16, tag=f'tD{i}', name=f'x3bf{i}') for i in range(NT)]
        with tc.tile_pool(name='p_f', bufs=1) as p_f:
            b2_sb = p_f.tile([1, D], F32)
            nc.sync.dma_start(out=b2_sb,
                              in_=bass.AP(vec_d['b2'], 0, [[0, 1], [1, D]]))
            bh_sb = p_f.tile([1, C], F32)
            nc.sync.dma_start(out=bh_sb,
                              in_=bass.AP(vec_d['bh'], 0, [[0, 1], [1, C]]))
            g1T = [p_f.tile([P, N], BF16, tag=f'g{fc}', name=f'g1T{fc}') for fc in range(FC)]
            for fc in range(FC):
                for nh in range(2):
                    pm = ps_ef.tile([P, 512], F32, tag='mm', bufs=3,
                                    name='pmF1')
                    for kc in range(DC):
                        nc.tensor.matmul(
                            pm, w1_bf[:, kc, fc * P:(fc + 1) * P],
                            xn2T[kc][nh],
                            start=(kc == 0), stop=(kc == DC - 1))
                    nc.scalar.activation(
                        out=g1T[fc][:, nh * 512:(nh + 1) * 512], in_=pm,
                        func=AF.Gelu_apprx_tanh, bias=b1T[:, fc:fc + 1],
                        scale=1.0)
            for i in range(NT):
                pm = ps_ef.tile([P, 512], F32, tag='mm', bufs=3, name='pmF2')
                for kc in range(FC):
                    nc.tensor.matmul(pm, g1T[kc][:, i * P:(i + 1) * P],
                                     w2_bf[:, kc, :],
                                     start=(kc == 0), stop=(kc == FC - 1))
                nc.vector.tensor_tensor(out=x3_bf[i], in0=pm,
                                        in1=x2[i], op=ALU.add)
            # mean pool over tokens
            pp = ps_ef.tile([1, D], F32, tag='sm', bufs=2, name='pp')
            for i in range(NT):
                nc.tensor.matmul(pp, ones_bf, x3_bf[i],
                                 start=(i == 0), stop=(i == NT - 1))
            pl = zp.tile([P, D], F32, tag='z', name='pl')
            pl = pl[0:1, :]
            nc.scalar.activation(out=pl, in_=pp, func=AF.Copy, scale=1.0 / N)
            nc.vector.tensor_tensor(out=pl, in0=pl, in1=b2_sb, op=ALU.add)
            # head layernorm on the pooled vector
            zh_full = ln_chunk(pl[0:1, :], eps_sb[0:1, :])
            zT_r = acts.tile([P, DC], F32R, tag='zT')
            for j in range(DC):
                pt = ps_ef.tile([P, 1], F32, tag='sm', bufs=2, name='pth')
                nc.tensor.transpose(pt, zh_full[0:1, j * P:(j + 1) * P],
                                    ident[0:1, 0:1])
                nc.any.tensor_scalar(
                    out=zT_r[:, j:j + 1], in0=pt,
                    scalar1=vec_pm['lnh_g'][:, j:j + 1],
                    scalar2=vec_pm['lnh_b'][:, j:j + 1],
                    op0=ALU.mult, op1=ALU.add)
            out_sb = p_f.tile([1, C], F32, tag='osb')
            for half in range(2):
                ph = ps_ef.tile([1, 500], F32, tag='sm', bufs=2, name='ph')
                for j in range(DC):
                    nc.tensor.matmul(
                        ph, zT_r[:, j:j + 1],
                        wh_r[:, j, half * 500:(half + 1) * 500],
                        start=(j == 0), stop=(j == DC - 1))
                nc.vector.tensor_tensor(
                    out=out_sb[:, half * 500:(half + 1) * 500], in0=ph,
                    in1=bh_sb[:, half * 500:(half + 1) * 500], op=ALU.add)
            nc.sync.dma_start(out=out_d[:], in_=out_sb)
        es_ps2.close()

    nc.finalize()
    return nc


_NC_CACHE = None


def kernel(**inputs) -> np.ndarray:
    global _NC_CACHE
    if _NC_CACHE is None:
        _NC_CACHE = build_bass()
    nc = _NC_CACHE

    arr = {k: np.ascontiguousarray(np.asarray(v, dtype=np.float32))
           for k, v in inputs.items()}
    x = arr.pop('x')                       # [8, 1024, 512]
    in_maps = [dict(arr, x=np.ascontiguousarray(x[i])) for i in range(N_CORES)]
    res = run_bass_kernel_spmd(nc, in_maps, core_ids=list(range(N_CORES)))
    return np.concatenate([res.results[i]['out'] for i in range(N_CORES)],
                          axis=0)


if __name__ == '__main__':
    rng = np.random.default_rng(0)
    s = lambda d: 1.0 / np.sqrt(d)
    ins = {
        'x': rng.standard_normal((8, N, D), dtype=np.float32),
        'mask': np.ones((N, 1), np.float32),
        'ln1_g': np.ones(D, np.float32), 'ln1_b': np.zeros(D, np.float32),
        'Wq': rng.standard_normal((D, D), dtype=np.float32) * s(D),
        'bq': np.zeros(D, np.float32),
        'Wk': rng.standard_normal((D, D), dtype=np.float32) * s(D),
        'bk': np.zeros(D, np.float32),
        'Wv': rng.standard_normal((D, D), dtype=np.float32) * s(D),
        'bv': np.zeros(D, np.float32),
        'Wo': rng.standard_normal((D, D), dtype=np.float32) * s(D),
        'bo': np.zeros(D, np.float32),
        'ln2_g': np.ones(D, np.float32), 'ln2_b': np.zeros(D, np.float32),
        'W1': rng.standard_normal((D, F), dtype=np.float32) * s(D),
        'b1': np.zeros(F, np.float32),
        'W2': rng.standard_normal((F, D), dtype=np.float32) * s(F),
        'b2': np.zeros(D, np.float32),
        'lnh_g': np.ones(D, np.float32), 'lnh_b': np.zeros(D, np.float32),
        'Wh': rng.standard_normal((D, C), dtype=np.float32) * s(D),
        'bh': np.zeros(C, np.float32),
    }
    out = kernel(**ins)
    print('out', out.shape, out.dtype, float(np.abs(out).max()))
